# revision 24
# baseline (speedup 1.0000x reference)
"""Trainium2 Bass kernel for the gnn_message_passing reward environment.

reference:
    diff   = feature - next_feature                    # [N, D]
    neigh  = next_action @ diff                        # [N, D]
    impact = (neigh @ neigh.T) / D                     # [N, N]
    normed = row_l2_normalize(next_feature)            # [N, D]
    sim    = normed @ normed.T                         # [N, N]
    out    = persona_a * next_action * sim             # reward_sim
           - persona_b * edges                         # reward_cost
           + persona_g * impact                        # reward_impact
    (persona_x = persona_t @ x, per-row scalars)

Distribution: 1D row shard across 8 NeuronCores (512 rows each).
Host precomputes diff (x16 fp8), next_action.T (fp8), normed.T (x16 fp8,
with persona_a folded into the row-sharded stationary copy), the mask
(x1/256 fp8) and the beta-scaled edge cost (bf16), so the device runs just
three row-sharded fp8 DoubleRow GEMMs:
  1. neighT[o] = diff.T @ A[o].T   (contraction over N, streamed chunks)
  2. sim rows  = nto.T @ ntf       (host-replicated right operand)
  3. impact    = neighT[o].T @ neighT (right operand from one fp8 AllGather
     that overlaps with GEMM 2 and its combine)
The elementwise reward combine is fused on DVE out of PSUM; the edge-cost
term folds in during phase 2 (under the AllGather). Output is bf16 (host
upcasts). DMA issue is spread across the SP/Activation queues to avoid
head-of-line blocking; the collective sits alone on the Pool queue. Reps
are software-pipelined: phase1 of rep k+1 is emitted between phase2(k) and
phase3(k) so it fills rep k's AllGather window and consecutive AllGathers
run back to back.
"""
import numpy as np
import ml_dtypes
from contextlib import ExitStack

import concourse.bass as bass
import concourse.tile as tile
from concourse import bacc, mybir
from concourse.bass_utils import run_bass_kernel_spmd

N = 4096          # graph nodes
D = 1024          # feature dim
NCORES = 8
R = N // NCORES   # 512 rows per core
RT = R // 128     # 4 row tiles per shard
DT = D // 128     # 8 d-tiles
KC = 4            # streamed k-chunks in GEMM 1 (8 k-tiles each)
KP = 4            # DoubleRow k-pairs per chunk
NB = N // 512     # 8 output column blocks

F32 = mybir.dt.float32
BF16 = mybir.dt.bfloat16
F8 = mybir.dt.float8e4
MUL = mybir.AluOpType.mult
ADD = mybir.AluOpType.add
DR = mybir.MatmulPerfMode.DoubleRow


def build(reps: int = 1, stage: int = 4, mock_cc: bool = False):
    nc = bacc.Bacc("TRN2", target_bir_lowering=False, debug=False,
                   num_devices=NCORES)

    diff = nc.dram_tensor("diff", [N, D], F8, kind="ExternalInput").ap()
    at = nc.dram_tensor("at", [N, R], F8, kind="ExternalInput").ap()
    nto = nc.dram_tensor("nto", [D, R], F8, kind="ExternalInput").ap()
    ntf = nc.dram_tensor("ntf", [D, N], F8, kind="ExternalInput").ap()
    ams = nc.dram_tensor("ams", [R, N], F8, kind="ExternalInput").ap()
    ed8 = nc.dram_tensor("ed8", [R, N], F8, kind="ExternalInput").ap()
    pbn = nc.dram_tensor("pbn", [128, RT], F32, kind="ExternalInput").ap()
    pgs = nc.dram_tensor("pgs", [128, RT], F32, kind="ExternalInput").ap()
    out = nc.dram_tensor("out", [R, N], BF16, kind="ExternalOutput").ap()

    rgroups = [list(range(NCORES))]

    def blk(ap):
        """[T*128, M] -> [128, T, M] partition-tiled view."""
        return ap.rearrange("(a p) m -> p a m", p=128)

    with tile.TileContext(nc) as tc, ExitStack() as ctx:
        const = ctx.enter_context(tc.tile_pool(name="const", bufs=1))
        own = ctx.enter_context(tc.tile_pool(name="own", bufs=3))
        stream = ctx.enter_context(tc.tile_pool(name="stream", bufs=1))
        outp_pool = ctx.enter_context(tc.tile_pool(name="outp", bufs=2))
        ps = ctx.enter_context(tc.tile_pool(name="ps", bufs=8, space="PSUM"))
        dram = ctx.enter_context(tc.tile_pool(name="dram", bufs=3, space="DRAM"))

        pbn_sb = const.tile([128, RT], F32)
        nc.sync.dma_start(pbn_sb[:], pbn[:])
        pgs_sb = const.tile([128, RT], F32)
        nc.sync.dma_start(pgs_sb[:], pgs[:])
        nto_sb = const.tile([128, DT, R], F8)
        nc.sync.dma_start(nto_sb[:], blk(nto))

        neighTs, ag_ins, ag_outs = {}, {}, {}

        def phase1(rep):
            """GEMM 1 (neighT = diff.T @ A_shard.T) + AG-input write."""
            ag_in = dram.tile([D, R], F8, name=f"ag_in{rep}", tag="agi")
            ag_out = dram.tile([NCORES, D, R], F8, addr_space="Shared",
                               name=f"ag_out{rep}", tag="ago")
            ag_ins[rep] = ag_in
            ag_outs[rep] = ag_out
            g1ps = []
            for d8 in range(DT):
                t = ps.tile([128, R], F32, name=f"g1ps{rep}_{d8}", tag="ps")
                g1ps.append(t)
            neighT = own.tile([128, DT, R], F8, name=f"neown{rep}",
                              tag="neown")
            neighTs[rep] = neighT
            for kc in range(KC):
                ksl = slice(kc * 1024, (kc + 1) * 1024)
                d_ch = stream.tile([128, 2 * KP, D], F8,
                                   name=f"d_ch{rep}_{kc}", tag="d_ch", bufs=2)
                a_ch = stream.tile([128, 2 * KP, R], F8,
                                   name=f"a_ch{rep}_{kc}", tag="a_ch", bufs=2)
                # GEMM 1 feeds the AllGather: its streams must not queue
                # behind the slack-tolerant phase-2 prefetches on the rings
                with tc.high_priority(offset=450):
                    nc.sync.dma_start(d_ch[:], blk(diff[ksl, :]))
                    nc.scalar.dma_start(a_ch[:], blk(at[ksl, :]))
                if kc < KC - 1:
                    for pr in range(KP):
                        for d8 in range(DT):
                            nc.tensor.matmul(
                                g1ps[d8][:],
                                d_ch[:, 2 * pr:2 * pr + 2,
                                     d8 * 128:(d8 + 1) * 128],
                                a_ch[:, 2 * pr:2 * pr + 2, :],
                                start=(kc == 0 and pr == 0), stop=False,
                                perf_mode=DR)
                else:
                    # finish banks one at a time; the fp8 copies pipeline
                    # under the remaining matmuls, then one AG-input write
                    for d8 in range(DT):
                        for pr in range(KP):
                            nc.tensor.matmul(
                                g1ps[d8][:],
                                d_ch[:, 2 * pr:2 * pr + 2,
                                     d8 * 128:(d8 + 1) * 128],
                                a_ch[:, 2 * pr:2 * pr + 2, :],
                                start=False, stop=(pr == KP - 1),
                                perf_mode=DR)
                        nc.scalar.copy(neighT[:, d8, :], g1ps[d8][:])
                    nc.sync.dma_start(blk(ag_in), neighT[:])

        def collective(rep):
            if mock_cc:
                nc.gpsimd.dma_start(ag_outs[rep][0][:], ag_ins[rep][:])
            else:
                nc.gpsimd.collective_compute(
                    "AllGather", mybir.AluOpType.bypass,
                    ins=[ag_ins[rep].opt()], outs=[ag_outs[rep].opt()],
                    replica_groups=rgroups)

        def phase2(rep, outps):
            """sim GEMM + mask + edge cost; fully under the AllGather."""
            outp = outp_pool.tile([128, RT, N], BF16, name=f"outp{rep}",
                                  tag="outp")
            outps[rep] = outp
            for g in range(NB // 2):
                gsl = slice(g * 1024, (g + 1) * 1024)
                ntf_g = stream.tile([128, DT, 1024], F8, name=f"ntf{rep}_{g}",
                                    tag="ntf_g", bufs=2)
                nc.sync.dma_start(ntf_g[:], blk(ntf)[:, :, gsl])
                ams_g = stream.tile([128, RT, 1024], F8, name=f"ams{rep}_{g}",
                                    tag="ams_g", bufs=2)
                nc.scalar.dma_start(ams_g[:], blk(ams[:, gsl]))
                ed8_g = stream.tile([128, RT, 1024], F8,
                                    name=f"ed8{rep}_{g}", tag="ed8_g", bufs=2)
                nc.scalar.dma_start(ed8_g[:], blk(ed8[:, gsl]))
                for b in range(2):
                    nb = 2 * g + b
                    csl = slice(nb * 512, (nb + 1) * 512)
                    bsl = slice(b * 512, (b + 1) * 512)
                    for mt in range(RT):
                        sps = ps.tile([128, 512], F32,
                                      name=f"sps{rep}_{nb}_{mt}", tag="ps")
                        for k2 in range(DT // 2):
                            nc.tensor.matmul(
                                sps[:],
                                nto_sb[:, 2 * k2:2 * k2 + 2,
                                       mt * 128:(mt + 1) * 128],
                                ntf_g[:, 2 * k2:2 * k2 + 2, bsl],
                                start=(k2 == 0), stop=(k2 == DT // 2 - 1),
                                perf_mode=DR)
                        # DVE only does the PSUM-sourced mask multiply; the
                        # edge scale + merge run on the otherwise-idle GPSIMD
                        # engine (walrus cannot lower STT or PSUM reads on
                        # Pool, hence two plain ops on SBUF operands)
                        edt = stream.tile([128, 512], BF16,
                                          name=f"edt{rep}_{nb}_{mt}",
                                          tag="edt", bufs=4)
                        nc.gpsimd.tensor_scalar(
                            edt[:], ed8_g[:, mt, bsl], pbn_sb[:, mt:mt + 1],
                            None, MUL)
                        sim_t = stream.tile([128, 512], BF16,
                                            name=f"sim{rep}_{nb}_{mt}",
                                            tag="sim_t", bufs=4)
                        nc.vector.tensor_tensor(
                            sim_t[:], sps[:], ams_g[:, mt, bsl], MUL)
                        nc.gpsimd.tensor_tensor(
                            outp[:, mt, csl], sim_t[:], edt[:], ADD)

        def phase3(rep, outps):
            """impact GEMM + final combine + output write.

            ner loads ride the Pool SWDGE ring: they are gated on the
            AllGather, and parking them on the HWDGE rings would
            head-of-line-block every later DMA behind the collective.
            """
            neighT = neighTs[rep]
            outp = outps[rep]
            for g in range(NB // 2):
                gsl = slice(g * 1024, (g + 1) * 1024)
                o_g = stream.tile([128, RT, 1024], BF16, name=f"o_g{rep}_{g}",
                                  tag="o_g", bufs=2)
                for b in range(2):
                    nb = 2 * g + b
                    csl = slice(nb * 512, (nb + 1) * 512)
                    bsl = slice(b * 512, (b + 1) * 512)
                    ner_b = stream.tile([128, DT, 512], F8,
                                        name=f"ner{rep}_{nb}", tag="ner_b",
                                        bufs=3)
                    nc.gpsimd.dma_start(ner_b[:], blk(ag_outs[rep][nb]))
                    for mt in range(RT):
                        ips = ps.tile([128, 512], F32,
                                      name=f"ips{rep}_{nb}_{mt}", tag="ps")
                        for k2 in range(DT // 2):
                            nc.tensor.matmul(
                                ips[:],
                                neighT[:, 2 * k2:2 * k2 + 2,
                                       mt * 128:(mt + 1) * 128],
                                ner_b[:, 2 * k2:2 * k2 + 2, :],
                                start=(k2 == 0), stop=(k2 == DT // 2 - 1),
                                perf_mode=DR)
                        nc.vector.scalar_tensor_tensor(
                            o_g[:, mt, bsl], ips[:], pgs_sb[:, mt:mt + 1],
                            outp[:, mt, csl], op0=MUL, op1=ADD)
                nc.sync.dma_start(blk(out[:, gsl]), o_g[:])

        outps = {}
        phase1(0)
        collective(0)
        if stage <= 1:
            neighT = neighTs[0]
            for d8 in range(DT):
                nc.gpsimd.dma_start(out[0:128, d8 * 512:(d8 + 1) * 512],
                                    neighT[:, d8, :])
        elif stage <= 3:
            phase2(0, outps)
            outp = outps[0]
            for mt in range(RT):
                nc.gpsimd.dma_start(out[mt * 128:(mt + 1) * 128, :],
                                    outp[:, mt, :])
        else:
            # Software pipeline, depth 2: rep k's AllGather window holds the
            # deferred phase3(k-1) (first: its ner loads are ready at window
            # start), phase2(k), and phase1(k+1) whose collective is queued
            # so consecutive AllGathers run back to back.
            for rep in range(reps):
                if rep >= 1:
                    phase3(rep - 1, outps)
                phase2(rep, outps)
                if rep + 1 < reps:
                    phase1(rep + 1)
                    collective(rep + 1)
            phase3(reps - 1, outps)

    nc.compile()
    return nc


_CACHE = {}


def _get_nc(reps=1, stage=4, mock_cc=False):
    key = (reps, stage, mock_cc)
    if key not in _CACHE:
        _CACHE[key] = build(reps, stage, mock_cc)
    return _CACHE[key]


def make_in_maps(feature, next_feature, next_action, edges, persona_t,
                 alpha, beta, gamma):
    F8NP = ml_dtypes.float8_e4m3
    BF16NP = ml_dtypes.bfloat16
    f = np.asarray(feature, dtype=np.float32)
    nf = np.asarray(next_feature, dtype=np.float32)
    A = np.asarray(next_action, dtype=np.float32)
    E = np.asarray(edges, dtype=np.float32)
    diff8 = ((f - nf) * 16.0).astype(F8NP)
    nrm = np.sqrt((nf * nf).sum(axis=1, keepdims=True))
    normed = nf / np.where(nrm > 0, nrm, 1.0)
    nrm2 = np.sqrt((normed * normed).sum(axis=1, keepdims=True))
    normed = normed / np.where(nrm2 > 0, nrm2, 1.0)
    nt16 = (normed * 16.0).astype(np.float32)
    ntf8 = np.ascontiguousarray(nt16.T).astype(F8NP)             # [D, N]
    at8 = np.ascontiguousarray(A.T).astype(F8NP)                 # [N, N]
    ams8 = (A * (1.0 / 256.0)).astype(F8NP)                      # exact
    ed8 = E.astype(F8NP)                                         # exact 0/1
    pt = np.asarray(persona_t, dtype=np.float32)
    pv_a = pt @ np.asarray(alpha, np.float32)                    # folded in nto
    pv_bn = -(pt @ np.asarray(beta, np.float32))
    pv_gs = (pt @ np.asarray(gamma, np.float32)) / (D * 256.0)
    nto_all = np.ascontiguousarray((nt16 * pv_a[:, None]).T)     # [D, N] f32

    def pcol(v, rs):
        return np.ascontiguousarray(v[rs].reshape(RT, 128).T)

    in_maps = []
    for c in range(NCORES):
        rs = slice(c * R, (c + 1) * R)
        in_maps.append({
            "diff": diff8,
            "at": at8[:, rs],
            "nto": nto_all[:, rs].astype(F8NP),
            "ntf": ntf8,
            "ams": ams8[rs],
            "ed8": ed8[rs],
            "pbn": pcol(pv_bn, rs),
            "pgs": pcol(pv_gs, rs),
        })
    return in_maps


def kernel(feature, next_feature, next_action, edges, persona_t,
           alpha, beta, gamma):
    nc = _get_nc(1)
    in_maps = make_in_maps(feature, next_feature, next_action, edges,
                           persona_t, alpha, beta, gamma)
    res = run_bass_kernel_spmd(nc, in_maps, list(range(NCORES)))
    return np.concatenate(
        [res.results[c]["out"].astype(np.float32) for c in range(NCORES)],
        axis=0)


# revision 25
# speedup vs baseline: 3.0520x; 3.0520x over previous
"""Trainium2 Bass kernel for the gnn_message_passing reward environment.

reference:
    diff   = feature - next_feature                    # [N, D]
    neigh  = next_action @ diff                        # [N, D]
    impact = (neigh @ neigh.T) / D                     # [N, N]
    normed = row_l2_normalize(next_feature)            # [N, D]
    sim    = normed @ normed.T                         # [N, N]
    out    = persona_a * next_action * sim             # reward_sim
           - persona_b * edges                         # reward_cost
           + persona_g * impact                        # reward_impact
    (persona_x = persona_t @ x, per-row scalars)

Distribution: 1D row shard across 8 NeuronCores (512 rows each).
Host precomputes diff (x16 fp8), next_action.T (fp8), normed.T (x16 fp8,
with persona_a folded into the row-sharded stationary copy), the mask
(x1/256 fp8) and the beta-scaled edge cost (bf16), so the device runs just
three row-sharded fp8 DoubleRow GEMMs:
  1. neighT[o] = diff.T @ A[o].T   (contraction over N, streamed chunks)
  2. sim rows  = nto.T @ ntf       (host-replicated right operand)
  3. impact    = neighT[o].T @ neighT (right operand from one fp8 AllGather
     that overlaps with GEMM 2 and its combine)
The elementwise reward combine is fused on DVE out of PSUM; the edge-cost
term folds in during phase 2 (under the AllGather). Output is bf16 (host
upcasts). DMA issue is spread across the SP/Activation queues to avoid
head-of-line blocking; the collective sits alone on the Pool queue. Reps
are software-pipelined: phase1 of rep k+1 is emitted between phase2(k) and
phase3(k) so it fills rep k's AllGather window and consecutive AllGathers
run back to back.
"""
import numpy as np
import ml_dtypes
from contextlib import ExitStack

import concourse.bass as bass
import concourse.tile as tile
from concourse import bacc, mybir
from concourse.bass_utils import run_bass_kernel_spmd

N = 4096          # graph nodes
D = 1024          # feature dim
NCORES = 8
R = N // NCORES   # 512 rows per core
RT = R // 128     # 4 row tiles per shard
DT = D // 128     # 8 d-tiles
KC = 4            # streamed k-chunks in GEMM 1 (8 k-tiles each)
KP = 4            # DoubleRow k-pairs per chunk
NB = N // 512     # 8 output column blocks

F32 = mybir.dt.float32
BF16 = mybir.dt.bfloat16
F8 = mybir.dt.float8e4
MUL = mybir.AluOpType.mult
ADD = mybir.AluOpType.add
DR = mybir.MatmulPerfMode.DoubleRow


def build(reps: int = 1, stage: int = 4, mock_cc: bool = False):
    nc = bacc.Bacc("TRN2", target_bir_lowering=False, debug=False,
                   num_devices=NCORES)

    diff = nc.dram_tensor("diff", [N, D], F8, kind="ExternalInput").ap()
    at = nc.dram_tensor("at", [N, R], F8, kind="ExternalInput").ap()
    nto = nc.dram_tensor("nto", [D, R], F8, kind="ExternalInput").ap()
    ntf = nc.dram_tensor("ntf", [D, N], F8, kind="ExternalInput").ap()
    ams = nc.dram_tensor("ams", [R, N], F8, kind="ExternalInput").ap()
    ed8 = nc.dram_tensor("ed8", [R, N], F8, kind="ExternalInput").ap()
    pbn = nc.dram_tensor("pbn", [128, RT], F32, kind="ExternalInput").ap()
    pgs = nc.dram_tensor("pgs", [128, RT], F32, kind="ExternalInput").ap()
    out = nc.dram_tensor("out", [R, N], BF16, kind="ExternalOutput").ap()

    rgroups = [list(range(NCORES))]

    def blk(ap):
        """[T*128, M] -> [128, T, M] partition-tiled view."""
        return ap.rearrange("(a p) m -> p a m", p=128)

    with tile.TileContext(nc) as tc, ExitStack() as ctx:
        const = ctx.enter_context(tc.tile_pool(name="const", bufs=1))
        own = ctx.enter_context(tc.tile_pool(name="own", bufs=3))
        stream = ctx.enter_context(tc.tile_pool(name="stream", bufs=1))
        outp_pool = ctx.enter_context(tc.tile_pool(name="outp", bufs=2))
        ps = ctx.enter_context(tc.tile_pool(name="ps", bufs=8, space="PSUM"))
        dram = ctx.enter_context(tc.tile_pool(name="dram", bufs=3, space="DRAM"))

        pbn_sb = const.tile([128, RT], F32)
        nc.sync.dma_start(pbn_sb[:], pbn[:])
        pgs_sb = const.tile([128, RT], F32)
        nc.sync.dma_start(pgs_sb[:], pgs[:])
        nto_sb = const.tile([128, DT, R], F8)
        nc.sync.dma_start(nto_sb[:], blk(nto))

        neighTs, ag_ins, ag_outs = {}, {}, {}

        def phase1(rep):
            """GEMM 1 (neighT = diff.T @ A_shard.T) + AG-input write."""
            ag_in = dram.tile([D, R], F8, name=f"ag_in{rep}", tag="agi")
            ag_out = dram.tile([NCORES, D, R], F8, addr_space="Shared",
                               name=f"ag_out{rep}", tag="ago")
            ag_ins[rep] = ag_in
            ag_outs[rep] = ag_out
            g1ps = []
            for d8 in range(DT):
                t = ps.tile([128, R], F32, name=f"g1ps{rep}_{d8}", tag="ps")
                g1ps.append(t)
            neighT = own.tile([128, DT, R], F8, name=f"neown{rep}",
                              tag="neown")
            neighTs[rep] = neighT
            for kc in range(KC):
                ksl = slice(kc * 1024, (kc + 1) * 1024)
                d_ch = stream.tile([128, 2 * KP, D], F8,
                                   name=f"d_ch{rep}_{kc}", tag="d_ch", bufs=2)
                a_ch = stream.tile([128, 2 * KP, R], F8,
                                   name=f"a_ch{rep}_{kc}", tag="a_ch", bufs=2)
                # GEMM 1 feeds the AllGather: its streams must not queue
                # behind the slack-tolerant phase-2 prefetches on the rings
                with tc.high_priority(offset=450):
                    nc.sync.dma_start(d_ch[:], blk(diff[ksl, :]))
                    nc.scalar.dma_start(a_ch[:], blk(at[ksl, :]))
                if kc < KC - 1:
                    for pr in range(KP):
                        for d8 in range(DT):
                            nc.tensor.matmul(
                                g1ps[d8][:],
                                d_ch[:, 2 * pr:2 * pr + 2,
                                     d8 * 128:(d8 + 1) * 128],
                                a_ch[:, 2 * pr:2 * pr + 2, :],
                                start=(kc == 0 and pr == 0), stop=False,
                                perf_mode=DR)
                else:
                    # finish banks one at a time; the fp8 copies pipeline
                    # under the remaining matmuls, then one AG-input write
                    for d8 in range(DT):
                        for pr in range(KP):
                            nc.tensor.matmul(
                                g1ps[d8][:],
                                d_ch[:, 2 * pr:2 * pr + 2,
                                     d8 * 128:(d8 + 1) * 128],
                                a_ch[:, 2 * pr:2 * pr + 2, :],
                                start=False, stop=(pr == KP - 1),
                                perf_mode=DR)
                        nc.scalar.copy(neighT[:, d8, :], g1ps[d8][:])
                    nc.sync.dma_start(blk(ag_in), neighT[:])

        def collective(rep):
            if mock_cc:
                nc.gpsimd.dma_start(ag_outs[rep][0][:], ag_ins[rep][:])
            else:
                nc.gpsimd.collective_compute(
                    "AllGather", mybir.AluOpType.bypass,
                    ins=[ag_ins[rep].opt()], outs=[ag_outs[rep].opt()],
                    replica_groups=rgroups)

        def phase2(rep, outps):
            """sim GEMM + mask + edge cost; fully under the AllGather."""
            outp = outp_pool.tile([128, RT, N], BF16, name=f"outp{rep}",
                                  tag="outp")
            outps[rep] = outp
            for g in range(NB // 2):
                gsl = slice(g * 1024, (g + 1) * 1024)
                ntf_g = stream.tile([128, DT, 1024], F8, name=f"ntf{rep}_{g}",
                                    tag="ntf_g", bufs=2)
                nc.sync.dma_start(ntf_g[:], blk(ntf)[:, :, gsl])
                ams_g = stream.tile([128, RT, 1024], F8, name=f"ams{rep}_{g}",
                                    tag="ams_g", bufs=2)
                nc.scalar.dma_start(ams_g[:], blk(ams[:, gsl]))
                ed8_g = stream.tile([128, RT, 1024], F8,
                                    name=f"ed8{rep}_{g}", tag="ed8_g", bufs=2)
                nc.scalar.dma_start(ed8_g[:], blk(ed8[:, gsl]))
                for b in range(2):
                    nb = 2 * g + b
                    csl = slice(nb * 512, (nb + 1) * 512)
                    bsl = slice(b * 512, (b + 1) * 512)
                    for mt in range(RT):
                        sps = ps.tile([128, 512], F32,
                                      name=f"sps{rep}_{nb}_{mt}", tag="ps")
                        for k2 in range(DT // 2):
                            nc.tensor.matmul(
                                sps[:],
                                nto_sb[:, 2 * k2:2 * k2 + 2,
                                       mt * 128:(mt + 1) * 128],
                                ntf_g[:, 2 * k2:2 * k2 + 2, bsl],
                                start=(k2 == 0), stop=(k2 == DT // 2 - 1),
                                perf_mode=DR)
                        nc.vector.tensor_tensor(
                            outp[:, mt, csl], sps[:], ams_g[:, mt, bsl], MUL)
                        nc.vector.scalar_tensor_tensor(
                            outp[:, mt, csl], ed8_g[:, mt, bsl],
                            pbn_sb[:, mt:mt + 1], outp[:, mt, csl],
                            op0=MUL, op1=ADD)

        def phase3(rep, outps):
            """impact GEMM + final combine + output write.

            ner loads ride the Pool SWDGE ring: they are gated on the
            AllGather, and parking them on the HWDGE rings would
            head-of-line-block every later DMA behind the collective.
            """
            neighT = neighTs[rep]
            outp = outps[rep]
            for g in range(NB // 2):
                gsl = slice(g * 1024, (g + 1) * 1024)
                o_g = stream.tile([128, RT, 1024], BF16, name=f"o_g{rep}_{g}",
                                  tag="o_g", bufs=2)
                for b in range(2):
                    nb = 2 * g + b
                    csl = slice(nb * 512, (nb + 1) * 512)
                    bsl = slice(b * 512, (b + 1) * 512)
                    ner_b = stream.tile([128, DT, 512], F8,
                                        name=f"ner{rep}_{nb}", tag="ner_b",
                                        bufs=3)
                    nc.gpsimd.dma_start(ner_b[:], blk(ag_outs[rep][nb]))
                    for mt in range(RT):
                        ips = ps.tile([128, 512], F32,
                                      name=f"ips{rep}_{nb}_{mt}", tag="ps")
                        for k2 in range(DT // 2):
                            nc.tensor.matmul(
                                ips[:],
                                neighT[:, 2 * k2:2 * k2 + 2,
                                       mt * 128:(mt + 1) * 128],
                                ner_b[:, 2 * k2:2 * k2 + 2, :],
                                start=(k2 == 0), stop=(k2 == DT // 2 - 1),
                                perf_mode=DR)
                        nc.vector.scalar_tensor_tensor(
                            o_g[:, mt, bsl], ips[:], pgs_sb[:, mt:mt + 1],
                            outp[:, mt, csl], op0=MUL, op1=ADD)
                nc.sync.dma_start(blk(out[:, gsl]), o_g[:])

        outps = {}
        phase1(0)
        collective(0)
        if stage <= 1:
            neighT = neighTs[0]
            for d8 in range(DT):
                nc.gpsimd.dma_start(out[0:128, d8 * 512:(d8 + 1) * 512],
                                    neighT[:, d8, :])
        elif stage <= 3:
            phase2(0, outps)
            outp = outps[0]
            for mt in range(RT):
                nc.gpsimd.dma_start(out[mt * 128:(mt + 1) * 128, :],
                                    outp[:, mt, :])
        else:
            # Software pipeline, depth 2: rep k's AllGather window holds the
            # deferred phase3(k-1) (first: its ner loads are ready at window
            # start), phase2(k), and phase1(k+1) whose collective is queued
            # so consecutive AllGathers run back to back.
            for rep in range(reps):
                if rep >= 1:
                    phase3(rep - 1, outps)
                phase2(rep, outps)
                if rep + 1 < reps:
                    phase1(rep + 1)
                    collective(rep + 1)
            phase3(reps - 1, outps)

    nc.compile()
    return nc


_CACHE = {}


def _get_nc(reps=1, stage=4, mock_cc=False):
    key = (reps, stage, mock_cc)
    if key not in _CACHE:
        _CACHE[key] = build(reps, stage, mock_cc)
    return _CACHE[key]


def make_in_maps(feature, next_feature, next_action, edges, persona_t,
                 alpha, beta, gamma):
    F8NP = ml_dtypes.float8_e4m3
    BF16NP = ml_dtypes.bfloat16
    f = np.asarray(feature, dtype=np.float32)
    nf = np.asarray(next_feature, dtype=np.float32)
    A = np.asarray(next_action, dtype=np.float32)
    E = np.asarray(edges, dtype=np.float32)
    diff8 = ((f - nf) * 16.0).astype(F8NP)
    nrm = np.sqrt((nf * nf).sum(axis=1, keepdims=True))
    normed = nf / np.where(nrm > 0, nrm, 1.0)
    nrm2 = np.sqrt((normed * normed).sum(axis=1, keepdims=True))
    normed = normed / np.where(nrm2 > 0, nrm2, 1.0)
    nt16 = (normed * 16.0).astype(np.float32)
    ntf8 = np.ascontiguousarray(nt16.T).astype(F8NP)             # [D, N]
    at8 = np.ascontiguousarray(A.T).astype(F8NP)                 # [N, N]
    ams8 = (A * (1.0 / 256.0)).astype(F8NP)                      # exact
    ed8 = E.astype(F8NP)                                         # exact 0/1
    pt = np.asarray(persona_t, dtype=np.float32)
    pv_a = pt @ np.asarray(alpha, np.float32)                    # folded in nto
    pv_bn = -(pt @ np.asarray(beta, np.float32))
    pv_gs = (pt @ np.asarray(gamma, np.float32)) / (D * 256.0)
    nto_all = np.ascontiguousarray((nt16 * pv_a[:, None]).T)     # [D, N] f32

    def pcol(v, rs):
        return np.ascontiguousarray(v[rs].reshape(RT, 128).T)

    in_maps = []
    for c in range(NCORES):
        rs = slice(c * R, (c + 1) * R)
        in_maps.append({
            "diff": diff8,
            "at": at8[:, rs],
            "nto": nto_all[:, rs].astype(F8NP),
            "ntf": ntf8,
            "ams": ams8[rs],
            "ed8": ed8[rs],
            "pbn": pcol(pv_bn, rs),
            "pgs": pcol(pv_gs, rs),
        })
    return in_maps


def kernel(feature, next_feature, next_action, edges, persona_t,
           alpha, beta, gamma):
    nc = _get_nc(1)
    in_maps = make_in_maps(feature, next_feature, next_action, edges,
                           persona_t, alpha, beta, gamma)
    res = run_bass_kernel_spmd(nc, in_maps, list(range(NCORES)))
    return np.concatenate(
        [res.results[c]["out"].astype(np.float32) for c in range(NCORES)],
        axis=0)


# revision 26
# speedup vs baseline: 3.1016x; 1.0162x over previous
"""Trainium2 Bass kernel for the gnn_message_passing reward environment.

reference:
    diff   = feature - next_feature                    # [N, D]
    neigh  = next_action @ diff                        # [N, D]
    impact = (neigh @ neigh.T) / D                     # [N, N]
    normed = row_l2_normalize(next_feature)            # [N, D]
    sim    = normed @ normed.T                         # [N, N]
    out    = persona_a * next_action * sim             # reward_sim
           - persona_b * edges                         # reward_cost
           + persona_g * impact                        # reward_impact
    (persona_x = persona_t @ x, per-row scalars)

Distribution: 1D row shard across 8 NeuronCores (512 rows each).
Host precomputes diff (x16 fp8), next_action.T (fp8), normed.T (x16 fp8,
with persona_a folded into the row-sharded stationary copy), the mask
(x1/256 fp8) and the beta-scaled edge cost (bf16), so the device runs just
three row-sharded fp8 DoubleRow GEMMs:
  1. neighT[o] = diff.T @ A[o].T   (contraction over N, streamed chunks)
  2. sim rows  = nto.T @ ntf       (host-replicated right operand)
  3. impact    = neighT[o].T @ neighT (right operand from one fp8 AllGather
     that overlaps with GEMM 2 and its combine)
The elementwise reward combine is fused on DVE out of PSUM; the edge-cost
term folds in during phase 2 (under the AllGather). Output is bf16 (host
upcasts). DMA issue is spread across the SP/Activation queues to avoid
head-of-line blocking; the collective sits alone on the Pool queue. Reps
are software-pipelined: phase1 of rep k+1 is emitted between phase2(k) and
phase3(k) so it fills rep k's AllGather window and consecutive AllGathers
run back to back.
"""
import numpy as np
import ml_dtypes
from contextlib import ExitStack

import concourse.bass as bass
import concourse.tile as tile
from concourse import bacc, mybir
from concourse.bass_utils import run_bass_kernel_spmd

N = 4096          # graph nodes
D = 1024          # feature dim
NCORES = 8
R = N // NCORES   # 512 rows per core
RT = R // 128     # 4 row tiles per shard
DT = D // 128     # 8 d-tiles
KC = 4            # streamed k-chunks in GEMM 1 (8 k-tiles each)
KP = 4            # DoubleRow k-pairs per chunk
NB = N // 512     # 8 output column blocks

F32 = mybir.dt.float32
BF16 = mybir.dt.bfloat16
F8 = mybir.dt.float8e4
MUL = mybir.AluOpType.mult
ADD = mybir.AluOpType.add
DR = mybir.MatmulPerfMode.DoubleRow


def build(reps: int = 1, stage: int = 4, mock_cc: bool = False):
    nc = bacc.Bacc("TRN2", target_bir_lowering=False, debug=False,
                   num_devices=NCORES)

    diff = nc.dram_tensor("diff", [N, D], F8, kind="ExternalInput").ap()
    at = nc.dram_tensor("at", [N, R], F8, kind="ExternalInput").ap()
    nto = nc.dram_tensor("nto", [D, R], F8, kind="ExternalInput").ap()
    ntf = nc.dram_tensor("ntf", [D, N], F8, kind="ExternalInput").ap()
    ams = nc.dram_tensor("ams", [R, N], F8, kind="ExternalInput").ap()
    ed8 = nc.dram_tensor("ed8", [R, N], F8, kind="ExternalInput").ap()
    pbn = nc.dram_tensor("pbn", [128, RT], F32, kind="ExternalInput").ap()
    pgs = nc.dram_tensor("pgs", [128, RT], F32, kind="ExternalInput").ap()
    out = nc.dram_tensor("out", [R, N], BF16, kind="ExternalOutput").ap()

    rgroups = [list(range(NCORES))]

    def blk(ap):
        """[T*128, M] -> [128, T, M] partition-tiled view."""
        return ap.rearrange("(a p) m -> p a m", p=128)

    with tile.TileContext(nc) as tc, ExitStack() as ctx:
        const = ctx.enter_context(tc.tile_pool(name="const", bufs=1))
        own = ctx.enter_context(tc.tile_pool(name="own", bufs=3))
        stream = ctx.enter_context(tc.tile_pool(name="stream", bufs=1))
        outp_pool = ctx.enter_context(tc.tile_pool(name="outp", bufs=2))
        ps = ctx.enter_context(tc.tile_pool(name="ps", bufs=8, space="PSUM"))
        dram = ctx.enter_context(tc.tile_pool(name="dram", bufs=3, space="DRAM"))

        pbn_sb = const.tile([128, RT], F32)
        nc.sync.dma_start(pbn_sb[:], pbn[:])
        pgs_sb = const.tile([128, RT], F32)
        nc.sync.dma_start(pgs_sb[:], pgs[:])
        nto_sb = const.tile([128, DT, R], F8)
        nc.sync.dma_start(nto_sb[:], blk(nto))

        neighTs, ag_ins, ag_outs = {}, {}, {}

        def phase1(rep):
            """GEMM 1 (neighT = diff.T @ A_shard.T) + AG-input write."""
            ag_in = dram.tile([D, R], F8, name=f"ag_in{rep}", tag="agi")
            ag_out = dram.tile([NCORES, D, R], F8, addr_space="Shared",
                               name=f"ag_out{rep}", tag="ago")
            ag_ins[rep] = ag_in
            ag_outs[rep] = ag_out
            g1ps = []
            for d8 in range(DT):
                t = ps.tile([128, R], F32, name=f"g1ps{rep}_{d8}", tag="ps")
                g1ps.append(t)
            neighT = own.tile([128, DT, R], F8, name=f"neown{rep}",
                              tag="neown")
            neighTs[rep] = neighT
            for kc in range(KC):
                ksl = slice(kc * 1024, (kc + 1) * 1024)
                d_ch = stream.tile([128, 2 * KP, D], F8,
                                   name=f"d_ch{rep}_{kc}", tag="d_ch", bufs=3)
                a_ch = stream.tile([128, 2 * KP, R], F8,
                                   name=f"a_ch{rep}_{kc}", tag="a_ch", bufs=3)
                # GEMM 1 feeds the AllGather: its streams must not queue
                # behind the slack-tolerant phase-2 prefetches on the rings
                with tc.high_priority(offset=450):
                    nc.sync.dma_start(d_ch[:], blk(diff[ksl, :]))
                    nc.scalar.dma_start(a_ch[:], blk(at[ksl, :]))
                if kc < KC - 1:
                    for pr in range(KP):
                        for d8 in range(DT):
                            nc.tensor.matmul(
                                g1ps[d8][:],
                                d_ch[:, 2 * pr:2 * pr + 2,
                                     d8 * 128:(d8 + 1) * 128],
                                a_ch[:, 2 * pr:2 * pr + 2, :],
                                start=(kc == 0 and pr == 0), stop=False,
                                perf_mode=DR)
                else:
                    # finish banks one at a time; the fp8 copies pipeline
                    # under the remaining matmuls, then one AG-input write
                    for d8 in range(DT):
                        for pr in range(KP):
                            nc.tensor.matmul(
                                g1ps[d8][:],
                                d_ch[:, 2 * pr:2 * pr + 2,
                                     d8 * 128:(d8 + 1) * 128],
                                a_ch[:, 2 * pr:2 * pr + 2, :],
                                start=False, stop=(pr == KP - 1),
                                perf_mode=DR)
                        nc.scalar.copy(neighT[:, d8, :], g1ps[d8][:])
                    nc.sync.dma_start(blk(ag_in), neighT[:])

        def collective(rep):
            if mock_cc:
                nc.gpsimd.dma_start(ag_outs[rep][0][:], ag_ins[rep][:])
            else:
                nc.gpsimd.collective_compute(
                    "AllGather", mybir.AluOpType.bypass,
                    ins=[ag_ins[rep].opt()], outs=[ag_outs[rep].opt()],
                    replica_groups=rgroups)

        def phase2(rep, outps):
            """sim GEMM + mask + edge cost; fully under the AllGather."""
            outp = outp_pool.tile([128, RT, N], BF16, name=f"outp{rep}",
                                  tag="outp")
            outps[rep] = outp
            for g in range(NB // 2):
                gsl = slice(g * 1024, (g + 1) * 1024)
                ntf_g = stream.tile([128, DT, 1024], F8, name=f"ntf{rep}_{g}",
                                    tag="ntf_g", bufs=3)
                nc.sync.dma_start(ntf_g[:], blk(ntf)[:, :, gsl])
                ams_g = stream.tile([128, RT, 1024], F8, name=f"ams{rep}_{g}",
                                    tag="ams_g", bufs=3)
                nc.scalar.dma_start(ams_g[:], blk(ams[:, gsl]))
                ed8_g = stream.tile([128, RT, 1024], F8,
                                    name=f"ed8{rep}_{g}", tag="ed8_g", bufs=3)
                nc.scalar.dma_start(ed8_g[:], blk(ed8[:, gsl]))
                for b in range(2):
                    nb = 2 * g + b
                    csl = slice(nb * 512, (nb + 1) * 512)
                    bsl = slice(b * 512, (b + 1) * 512)
                    for mt in range(RT):
                        sps = ps.tile([128, 512], F32,
                                      name=f"sps{rep}_{nb}_{mt}", tag="ps")
                        for k2 in range(DT // 2):
                            nc.tensor.matmul(
                                sps[:],
                                nto_sb[:, 2 * k2:2 * k2 + 2,
                                       mt * 128:(mt + 1) * 128],
                                ntf_g[:, 2 * k2:2 * k2 + 2, bsl],
                                start=(k2 == 0), stop=(k2 == DT // 2 - 1),
                                perf_mode=DR)
                        nc.vector.tensor_tensor(
                            outp[:, mt, csl], sps[:], ams_g[:, mt, bsl], MUL)
                        nc.vector.scalar_tensor_tensor(
                            outp[:, mt, csl], ed8_g[:, mt, bsl],
                            pbn_sb[:, mt:mt + 1], outp[:, mt, csl],
                            op0=MUL, op1=ADD)

        def phase3(rep, outps):
            """impact GEMM + final combine + output write.

            ner loads ride the Pool SWDGE ring: they are gated on the
            AllGather, and parking them on the HWDGE rings would
            head-of-line-block every later DMA behind the collective.
            """
            neighT = neighTs[rep]
            outp = outps[rep]
            for g in range(NB // 2):
                gsl = slice(g * 1024, (g + 1) * 1024)
                o_g = stream.tile([128, RT, 1024], BF16, name=f"o_g{rep}_{g}",
                                  tag="o_g", bufs=3)
                for b in range(2):
                    nb = 2 * g + b
                    csl = slice(nb * 512, (nb + 1) * 512)
                    bsl = slice(b * 512, (b + 1) * 512)
                    ner_b = stream.tile([128, DT, 512], F8,
                                        name=f"ner{rep}_{nb}", tag="ner_b",
                                        bufs=4)
                    nc.gpsimd.dma_start(ner_b[:], blk(ag_outs[rep][nb]))
                    for mt in range(RT):
                        ips = ps.tile([128, 512], F32,
                                      name=f"ips{rep}_{nb}_{mt}", tag="ps")
                        for k2 in range(DT // 2):
                            nc.tensor.matmul(
                                ips[:],
                                neighT[:, 2 * k2:2 * k2 + 2,
                                       mt * 128:(mt + 1) * 128],
                                ner_b[:, 2 * k2:2 * k2 + 2, :],
                                start=(k2 == 0), stop=(k2 == DT // 2 - 1),
                                perf_mode=DR)
                        nc.vector.scalar_tensor_tensor(
                            o_g[:, mt, bsl], ips[:], pgs_sb[:, mt:mt + 1],
                            outp[:, mt, csl], op0=MUL, op1=ADD)
                nc.sync.dma_start(blk(out[:, gsl]), o_g[:])

        outps = {}
        phase1(0)
        collective(0)
        if stage <= 1:
            neighT = neighTs[0]
            for d8 in range(DT):
                nc.gpsimd.dma_start(out[0:128, d8 * 512:(d8 + 1) * 512],
                                    neighT[:, d8, :])
        elif stage <= 3:
            phase2(0, outps)
            outp = outps[0]
            for mt in range(RT):
                nc.gpsimd.dma_start(out[mt * 128:(mt + 1) * 128, :],
                                    outp[:, mt, :])
        else:
            # Software pipeline, depth 2: rep k's AllGather window holds the
            # deferred phase3(k-1) (first: its ner loads are ready at window
            # start), phase2(k), and phase1(k+1) whose collective is queued
            # so consecutive AllGathers run back to back.
            for rep in range(reps):
                if rep >= 1:
                    phase3(rep - 1, outps)
                phase2(rep, outps)
                if rep + 1 < reps:
                    phase1(rep + 1)
                    collective(rep + 1)
            phase3(reps - 1, outps)

    nc.compile()
    return nc


_CACHE = {}


def _get_nc(reps=1, stage=4, mock_cc=False):
    key = (reps, stage, mock_cc)
    if key not in _CACHE:
        _CACHE[key] = build(reps, stage, mock_cc)
    return _CACHE[key]


def make_in_maps(feature, next_feature, next_action, edges, persona_t,
                 alpha, beta, gamma):
    F8NP = ml_dtypes.float8_e4m3
    BF16NP = ml_dtypes.bfloat16
    f = np.asarray(feature, dtype=np.float32)
    nf = np.asarray(next_feature, dtype=np.float32)
    A = np.asarray(next_action, dtype=np.float32)
    E = np.asarray(edges, dtype=np.float32)
    diff8 = ((f - nf) * 16.0).astype(F8NP)
    nrm = np.sqrt((nf * nf).sum(axis=1, keepdims=True))
    normed = nf / np.where(nrm > 0, nrm, 1.0)
    nrm2 = np.sqrt((normed * normed).sum(axis=1, keepdims=True))
    normed = normed / np.where(nrm2 > 0, nrm2, 1.0)
    nt16 = (normed * 16.0).astype(np.float32)
    ntf8 = np.ascontiguousarray(nt16.T).astype(F8NP)             # [D, N]
    at8 = np.ascontiguousarray(A.T).astype(F8NP)                 # [N, N]
    ams8 = (A * (1.0 / 256.0)).astype(F8NP)                      # exact
    ed8 = E.astype(F8NP)                                         # exact 0/1
    pt = np.asarray(persona_t, dtype=np.float32)
    pv_a = pt @ np.asarray(alpha, np.float32)                    # folded in nto
    pv_bn = -(pt @ np.asarray(beta, np.float32))
    pv_gs = (pt @ np.asarray(gamma, np.float32)) / (D * 256.0)
    nto_all = np.ascontiguousarray((nt16 * pv_a[:, None]).T)     # [D, N] f32

    def pcol(v, rs):
        return np.ascontiguousarray(v[rs].reshape(RT, 128).T)

    in_maps = []
    for c in range(NCORES):
        rs = slice(c * R, (c + 1) * R)
        in_maps.append({
            "diff": diff8,
            "at": at8[:, rs],
            "nto": nto_all[:, rs].astype(F8NP),
            "ntf": ntf8,
            "ams": ams8[rs],
            "ed8": ed8[rs],
            "pbn": pcol(pv_bn, rs),
            "pgs": pcol(pv_gs, rs),
        })
    return in_maps


def kernel(feature, next_feature, next_action, edges, persona_t,
           alpha, beta, gamma):
    nc = _get_nc(1)
    in_maps = make_in_maps(feature, next_feature, next_action, edges,
                           persona_t, alpha, beta, gamma)
    res = run_bass_kernel_spmd(nc, in_maps, list(range(NCORES)))
    return np.concatenate(
        [res.results[c]["out"].astype(np.float32) for c in range(NCORES)],
        axis=0)
